# revision 1
# baseline (speedup 1.0000x reference)
"""Trainium2 Bass kernel for nn_BaseGenerator_71451075936296.

6-layer post-norm dense transformer (B=32, S=256, E=1024, H=16, F=4096,
V=192) with a per-head additive attention bias gathered from distance /
isopen embedding tables.

Strategy: data-parallel over batch across 8 NeuronCores (4 sequences =
1024 tokens per core), weights replicated. All GEMMs run in bf16 on the
TensorEngine with fp32 PSUM accumulation; layernorm / softmax statistics
stay in fp32. Activations live in SBUF for the whole forward pass.

Host-side prep is limited to layout work: weight transposes / bf16
casts / tiling, index dtype casts, and building the additive attention
bias tensor (embedding-table lookups + causal/pad masking) which the
spec's sharding hint treats as a replicated input tensor. The token
embedding gather runs on-device via indirect DMA.
"""

import math
from contextlib import ExitStack

import numpy as np
import ml_dtypes

import concourse.bass as bass
import concourse.mybir as mybir
import concourse.tile as tile
from concourse import bacc
from concourse.bass_utils import run_bass_kernel_spmd
from concourse.masks import make_identity

B, S, E, H, F, L, V = 32, 256, 1024, 16, 4096, 6, 192
DH = E // H          # 64
NCORES = 8
BL = B // NCORES     # 4 sequences per core
T = BL * S           # 1024 tokens per core
P = 128
NT = T // P          # 8 token tiles
NE = E // P          # 8 E chunks
NF = F // P          # 32 F chunks
EPS = 1e-5
NEG = -1e30

bf16 = mybir.dt.bfloat16
f32 = mybir.dt.float32
i32 = mybir.dt.int32
AF = mybir.ActivationFunctionType
OP = mybir.AluOpType

nbf16 = ml_dtypes.bfloat16

# swapped to AF.Identity by test_sim.py (CoreSim lacks Gelu); HW uses Gelu
GELU_FUNC = AF.Gelu
XT_DMA = False   # x->xT transposes via DMA engine instead of PE
PT_DMA = False   # attention p transposes via DMA engine instead of PE
STOP_AFTER = None  # debug: "qk" | "v" | "attn" | "wo" | "h" | "ffn"
ATTN_SUB = 3   # debug: 0=scores 1=+softmax 2=+transpose 3=full
SC_VARIANT = 0


def _emit(ctx, tc, d, layers):
    nc = tc.nc
    X = mybir.AxisListType.X

    pool = lambda name, bufs, **kw: ctx.enter_context(
        tc.tile_pool(name=name, bufs=bufs, **kw))

    const = pool("const", 1)
    ident = const.tile([P, P], bf16)
    make_identity(nc, ident)
    ones_row = const.tile([1, P], bf16)
    nc.vector.memset(ones_row, 1.0)
    eps_t = const.tile([P, 1], f32)
    nc.vector.memset(eps_t, EPS)

    # persistent state tiles (allocated once, updated in place per layer)
    big = pool("big", 1)
    x_t = [big.tile([P, E], bf16, tag=f"x{t}", name=f"x{t}") for t in range(NT)]
    xT = big.tile([P, NE, T], bf16, tag="xT", name="xT")          # [feat%128, feat//128, tok]
    qk_t = [big.tile([P, T], bf16, tag=f"qk{m}", name=f"qk{m}") for m in range(16)]
    v_t = [big.tile([P, E], bf16, tag=f"v{t}", name=f"v{t}") for t in range(NT)]
    ao_t = [big.tile([P, T], bf16, tag=f"ao{e}", name=f"ao{e}") for e in range(NE)]
    h_t = [big.tile([P, T // 2], bf16, tag=f"h{m}", name=f"h{m}") for m in range(NF)]
    y_t = [big.tile([P, E], bf16, tag=f"y{t}", name=f"y{t}") for t in range(NT // 2)]

    ps = pool("ps", 8, space="PSUM")
    wp = pool("wp", 3)        # [P,1024]bf16 all-K weight stripes (qk / w1)
    wsp = pool("wsp", 3)      # [P,1024]bf16 streamed rhs stripes (v / wo)
    w2p = pool("w2p", 3)      # [P,512]bf16 w2 stripes
    wgp = pool("wgp", 8)      # [P,V]bf16 logit stripes
    bp = pool("bp", 4)        # bias tiles [P,2,256]bf16 (head pair)
    sp = pool("sp", 6)        # softmax exp tiles [P,256]bf16
    pp = pool("pp", 4)        # pT tiles [P,2,2,P]bf16
    tmp = pool("tmp", 2)      # f32 [P,1024]
    st = pool("st", 8)        # small stats
    lnp = pool("lnp", 2)      # replicated ln vecs f32 [P,E]
    colp = pool("colp", 2)    # per-layer bias column tiles
    rowp = pool("rowp", 2)    # [1,E] bias rows

    def psum(shape, dt=f32):
        return ps.tile(shape, dt, tag="ps", name="ps")

    def dma(out, in_):
        nc.sync.dma_start(out=out, in_=in_)

    def row_ap(ap1d):
        return ap1d.rearrange("(o e) -> o e", o=1)

    def col_ap(ap1d):
        return ap1d.rearrange("(p o) -> p o", o=1)

    def ln_inplace(t, xsum, s_rep, b_rep):
        """x_t[t] = LN(xsum) * s + b.  xsum: f32 [P,E] tile (clobbered)."""
        stats = st.tile([P, 2, 6], f32, tag="bnst", name="bnst")
        for sg in range(2):
            nc.vector.bn_stats(out=stats[:, sg, :], in_=xsum[:, sg * 512:(sg + 1) * 512])
        mv = st.tile([P, 2], f32, tag="bnmv", name="bnmv")
        nc.vector.bn_aggr(out=mv, in_=stats)
        std = st.tile([P, 1], f32, tag="bnsd", name="bnsd")
        nc.scalar.activation(out=std, in_=mv[:, 1:2], func=AF.Sqrt, bias=eps_t, scale=1.0)
        rstd = st.tile([P, 1], f32, tag="bnrs", name="bnrs")
        nc.vector.reciprocal(out=rstd, in_=std)
        nc.vector.scalar_tensor_tensor(
            out=xsum, in0=xsum, scalar=mv[:, 0:1], in1=s_rep,
            op0=OP.subtract, op1=OP.mult)
        nc.vector.scalar_tensor_tensor(
            out=x_t[t], in0=xsum, scalar=rstd, in1=b_rep,
            op0=OP.mult, op1=OP.add)

    def load_ln(s_ap, b_ap):
        s_rep = lnp.tile([P, E], f32, tag="lns", name="lns")
        b_rep = lnp.tile([P, E], f32, tag="lnb", name="lnb")
        nc.gpsimd.dma_start(out=s_rep, in_=s_ap.to_broadcast([P, E]))
        nc.gpsimd.dma_start(out=b_rep, in_=b_ap.to_broadcast([P, E]))
        return s_rep, b_rep

    def transpose_x_to_xT():
        for t in range(NT):
            if XT_DMA:
                nc.sync.dma_start_transpose(
                    out=xT[:, :, t * P:(t + 1) * P], in_=x_t[t][:])
            else:
                for e in range(NE):
                    pt = psum([P, P], bf16)
                    nc.tensor.transpose(
                        out=pt, in_=x_t[t][:, e * P:(e + 1) * P], identity=ident)
                    nc.any.tensor_copy(out=xT[:, e, t * P:(t + 1) * P], in_=pt)

    # ---- embedding: gather tok32 rows by token id, add positional ----
    for t in range(NT):
        offs = st.tile([P, 1], i32, tag="offs", name="offs")
        dma(offs, col_ap(d["seq"].ap()[t * P:(t + 1) * P]))
        g = tmp.tile([P, E], f32, tag="tmp", name="gtmp")
        nc.gpsimd.indirect_dma_start(
            out=g[:], out_offset=None, in_=d["tok32"].ap(),
            in_offset=bass.IndirectOffsetOnAxis(ap=offs[:, :1], axis=0))
        pos = tmp.tile([P, E], f32, tag="tmp", name="ptmp")
        sl = (t % 2) * P
        dma(pos, d["pos"].ap()[sl:sl + P, :])
        nc.vector.tensor_add(out=x_t[t], in0=g, in1=pos)

    # ---- layers ----
    for l in range(layers):
        transpose_x_to_xT()
        bqk_c = colp.tile([P, 16], f32, tag="bqk", name="bqkc")
        dma(bqk_c, d["bqk"].ap()[l])
        bv_c = colp.tile([P, NE], f32, tag="bv", name="bvc")
        dma(bv_c, d["bv"].ap()[l])
        b1_c = colp.tile([P, NF], f32, tag="b1", name="b1c")
        dma(b1_c, d["b1"].ap()[l])

        # --- q,k projections: feature-major [feat, T], weights stationary ---
        for m in range(16):
            wt = wp.tile([P, NE * P], bf16, tag="wp", name="wqk")
            dma(wt, d["wqk"].ap()[l, m])
            pse = [psum([P, 512]) for _ in range(2)]
            for k in range(NE):
                for hf in range(2):
                    nc.tensor.matmul(
                        out=pse[hf], lhsT=wt[:, k * P:(k + 1) * P],
                        rhs=xT[:, k, hf * 512:(hf + 1) * 512],
                        start=(k == 0), stop=(k == NE - 1))
            for hf in range(2):
                nc.scalar.activation(
                    out=qk_t[m][:, hf * 512:(hf + 1) * 512], in_=pse[hf],
                    func=AF.Identity, bias=bqk_c[:, m:m + 1],
                    scale=0.125 if m < 8 else 1.0)

        # --- v projection: token-major [tok, feat], activations stationary ---
        for tq in range(2):
            pse = [[psum([P, 512]) for _ in range(2)] for _ in range(4)]
            for k in range(NE):
                wv = wsp.tile([P, E], bf16, tag="wsp", name="wv")
                dma(wv, d["wv"].ap()[l, k])
                for t4 in range(4):
                    tt = tq * 4 + t4
                    for hf in range(2):
                        nc.tensor.matmul(
                            out=pse[t4][hf], lhsT=xT[:, k, tt * P:(tt + 1) * P],
                            rhs=wv[:, hf * 512:(hf + 1) * 512],
                            start=(k == 0), stop=(k == NE - 1))
            for t4 in range(4):
                for hf in range(2):
                    nc.any.tensor_copy(
                        out=v_t[tq * 4 + t4][:, hf * 512:(hf + 1) * 512],
                        in_=pse[t4][hf])

        # --- attention: per sequence b, head hd ---
        for b in range(BL):
            for jj in range(H // 2):
                bias2 = bp.tile([P, 2, 2, 256], bf16, tag="bp", name="bias2")
                for qt in range(2):
                    dma(bias2[:, qt], d["bias"].ap()[b, qt, :, 2 * jj:2 * jj + 2, :])
                for hh in range(2):
                    hd = 2 * jj + hh
                    m, r = hd // 2, (hd % 2) * DH
                    pT = pp.tile([P, 2, 2, P], bf16, tag="pp", name="pT")
                    for qt in range(2):
                        sc = psum([P, 256])
                        nc.tensor.matmul(
                            out=sc,
                            lhsT=qk_t[m][r:r + DH, b * 256 + qt * P: b * 256 + qt * P + P],
                            rhs=qk_t[8 + m][r:r + DH, b * 256:(b + 1) * 256],
                            start=True, stop=False)
                        nc.tensor.matmul(
                            out=sc, lhsT=ident, rhs=bias2[:, qt, hh],
                            start=False, stop=True)
                        e_sb = sp.tile([P, 256], bf16, tag="sp", name="esb")
                        ssum = st.tile([P, 1], f32, tag="ssum", name="ssum")
                        nc.scalar.activation(out=e_sb, in_=sc, func=AF.Exp,
                                             accum_out=ssum)
                        rinv = st.tile([P, 1], f32, tag="rinv", name="rinv")
                        nc.vector.reciprocal(out=rinv, in_=ssum)
                        nc.vector.tensor_scalar_mul(out=e_sb, in0=e_sb, scalar1=rinv)
                        if PT_DMA:
                            nc.sync.dma_start_transpose(out=pT[:, :, qt, :], in_=e_sb[:])
                        else:
                            for kb in range(2):
                                pt = psum([P, P], bf16)
                                nc.tensor.transpose(
                                    out=pt, in_=e_sb[:, kb * P:(kb + 1) * P],
                                    identity=ident)
                                nc.vector.tensor_copy(out=pT[:, kb, qt, :], in_=pt)
                    ot = psum([DH, 256])
                    for kb in range(2):
                        nc.tensor.matmul(
                            out=ot, lhsT=v_t[b * 2 + kb][:, hd * DH:(hd + 1) * DH],
                            rhs=pT[:, kb].rearrange("p a b -> p (a b)"),
                            start=(kb == 0), stop=(kb == 1))
                    nc.scalar.activation(
                        out=ao_t[m][r:r + DH, b * 256:(b + 1) * 256], in_=ot,
                        func=AF.Identity, bias=bv_c[r:r + DH, m:m + 1], scale=1.0)

        # --- Wo + residual + LN1 ---
        ln1 = load_ln(row_ap(d["ln1s"].ap()[l]), row_ap(d["ln1b"].ap()[l]))
        bo_row = rowp.tile([1, E], bf16, tag="row", name="borow")
        dma(bo_row, row_ap(d["bo"].ap()[l]))
        for tq in range(2):
            pse = [[psum([P, 512]) for _ in range(2)] for _ in range(4)]
            for k in range(NE):
                wo = wsp.tile([P, E], bf16, tag="wsp", name="wo")
                dma(wo, d["wo"].ap()[l, k])
                for t4 in range(4):
                    tt = tq * 4 + t4
                    for hf in range(2):
                        nc.tensor.matmul(
                            out=pse[t4][hf], lhsT=ao_t[k][:, tt * P:(tt + 1) * P],
                            rhs=wo[:, hf * 512:(hf + 1) * 512],
                            start=(k == 0), stop=False)
            for t4 in range(4):
                tt = tq * 4 + t4
                for hf in range(2):
                    nc.tensor.matmul(
                        out=pse[t4][hf], lhsT=ones_row,
                        rhs=bo_row[:, hf * 512:(hf + 1) * 512], start=False, stop=True)
                xsum = tmp.tile([P, E], f32, tag="tmp", name="xsum1")
                for hf in range(2):
                    nc.vector.tensor_add(
                        out=xsum[:, hf * 512:(hf + 1) * 512], in0=pse[t4][hf],
                        in1=x_t[tt][:, hf * 512:(hf + 1) * 512])
                ln_inplace(tt, xsum, *ln1)

        # --- FFN ---
        transpose_x_to_xT()
        ln2 = load_ln(row_ap(d["ln2s"].ap()[l]), row_ap(d["ln2b"].ap()[l]))
        b2_row = rowp.tile([1, E], bf16, tag="row", name="b2row")
        dma(b2_row, row_ap(d["b2"].ap()[l]))
        for th in range(2):  # T halves
            for m in range(NF):
                wt = wp.tile([P, NE * P], bf16, tag="wp", name="w1t")
                dma(wt, d["w1"].ap()[l, m])
                ph = psum([P, 512])
                for k in range(NE):
                    nc.tensor.matmul(
                        out=ph, lhsT=wt[:, k * P:(k + 1) * P],
                        rhs=xT[:, k, th * 512:(th + 1) * 512],
                        start=(k == 0), stop=(k == NE - 1))
                nc.scalar.activation(out=h_t[m], in_=ph, func=GELU_FUNC,
                                     bias=b1_c[:, m:m + 1], scale=1.0)
            for eh in range(2):  # E halves of FFN output
                pys = [psum([P, 512]) for _ in range(NT // 2)]
                for k in range(NF):
                    w2s = w2p.tile([P, 512], bf16, tag="w2p", name="w2s")
                    dma(w2s, d["w2"].ap()[l, k, :, eh * 512:(eh + 1) * 512])
                    for t4 in range(NT // 2):
                        nc.tensor.matmul(
                            out=pys[t4], lhsT=h_t[k][:, t4 * P:(t4 + 1) * P],
                            rhs=w2s, start=(k == 0), stop=False)
                for t4 in range(NT // 2):
                    nc.tensor.matmul(
                        out=pys[t4], lhsT=ones_row,
                        rhs=b2_row[:, eh * 512:(eh + 1) * 512], start=False, stop=True)
                    nc.any.tensor_copy(out=y_t[t4][:, eh * 512:(eh + 1) * 512],
                                       in_=pys[t4])
            for t4 in range(NT // 2):
                tt = th * (NT // 2) + t4
                xsum = tmp.tile([P, E], f32, tag="tmp", name="xsum2")
                nc.vector.tensor_add(out=xsum, in0=y_t[t4], in1=x_t[tt])
                ln_inplace(tt, xsum, *ln2)

    # ---- final LN + logits ----
    lnf = load_ln(row_ap(d["lnfs"].ap()), row_ap(d["lnfb"].ap()))
    for t in range(NT):
        xsum = tmp.tile([P, E], f32, tag="tmp", name="xsumf")
        nc.vector.tensor_copy(out=xsum, in_=x_t[t])
        ln_inplace(t, xsum, *lnf)
    transpose_x_to_xT()
    wgs = []
    for k in range(NE):
        wg = wgp.tile([P, V], bf16, tag="wg", name="wg")
        dma(wg, d["wg"].ap()[k])
        wgs.append(wg)
    bg_row = rowp.tile([1, V], bf16, tag="rowg", name="bgrow")
    dma(bg_row, row_ap(d["bg"].ap()))
    for t in range(NT):
        pl = psum([P, V])
        for k in range(NE):
            nc.tensor.matmul(out=pl, lhsT=xT[:, k, t * P:(t + 1) * P], rhs=wgs[k],
                             start=(k == 0), stop=False)
        nc.tensor.matmul(out=pl, lhsT=ones_row, rhs=bg_row, start=False, stop=True)
        lo = tmp.tile([P, V], f32, tag="lo", name="lo")
        nc.any.tensor_copy(out=lo, in_=pl)
        dma(d["out"].ap()[t * P:(t + 1) * P, :], lo)


def _declare(nc):
    d = {}
    def inp(name, shape, dt):
        d[name] = nc.dram_tensor(name, list(shape), dt, kind="ExternalInput")
    inp("seq", [T], i32)
    inp("tok32", [V, E], f32)
    inp("pos", [S, E], f32)
    inp("bias", [BL, 2, P, H, S], bf16)
    inp("wqk", [L, 16, P, NE * P], bf16)
    inp("wv", [L, NE, P, E], bf16)
    inp("bqk", [L, P, 16], f32)
    inp("bv", [L, P, NE], f32)
    inp("wo", [L, NE, P, E], bf16)
    inp("bo", [L, E], bf16)
    inp("w1", [L, NF, P, NE * P], bf16)
    inp("b1", [L, P, NF], f32)
    inp("w2", [L, NF, P, E], bf16)
    inp("b2", [L, E], bf16)
    inp("ln1s", [L, E], f32)
    inp("ln1b", [L, E], f32)
    inp("ln2s", [L, E], f32)
    inp("ln2b", [L, E], f32)
    inp("lnfs", [E], f32)
    inp("lnfb", [E], f32)
    inp("wg", [NE, P, V], bf16)
    inp("bg", [V], bf16)
    d["out"] = nc.dram_tensor("out", [T, V], f32, kind="ExternalOutput")
    return d


_BUILT = {}


def build(layers=L):
    key = ("nc", layers, str(GELU_FUNC), XT_DMA, PT_DMA)
    if key in _BUILT:
        return _BUILT[key]
    nc = bacc.Bacc("TRN2", target_bir_lowering=False, debug=False)
    d = _declare(nc)
    with tile.TileContext(nc) as tc:
        with ExitStack() as ctx:
            _emit(ctx, tc, d, layers)
    nc.compile()
    _BUILT[key] = nc
    return nc


def prep_shared(inputs):
    g = lambda k: np.asarray(inputs[k])
    sh = {}
    sh["tok32"] = (g("tok_emb") * math.sqrt(E)).astype(np.float32)
    sh["pos"] = g("pos_emb").astype(np.float32)

    WqkvT = np.ascontiguousarray(g("Wqkv").transpose(0, 2, 1)).astype(np.float32)  # [L,E,3E]
    qk = WqkvT[:, :, :2 * E].reshape(L, NE, P, 16, P).transpose(0, 3, 2, 1, 4)
    sh["wqk"] = np.ascontiguousarray(qk.reshape(L, 16, P, NE * P)).astype(nbf16)
    sh["wv"] = np.ascontiguousarray(WqkvT[:, :, 2 * E:].reshape(L, NE, P, E)).astype(nbf16)
    bqkv = g("bqkv").astype(np.float32)
    bqk = bqkv[:, :2 * E].copy()
    bqk[:, :E] *= 0.125
    sh["bqk"] = np.ascontiguousarray(bqk.reshape(L, 16, P).transpose(0, 2, 1))
    sh["bv"] = np.ascontiguousarray(
        bqkv[:, 2 * E:].reshape(L, NE, P).transpose(0, 2, 1))

    WoT = g("Wo").transpose(0, 2, 1)
    sh["wo"] = np.ascontiguousarray(WoT.reshape(L, NE, P, E)).astype(nbf16)
    sh["bo"] = g("bo").astype(nbf16)

    W1T = g("W1").transpose(0, 2, 1)  # [L,E,F]
    w1 = W1T.reshape(L, NE, P, NF, P).transpose(0, 3, 2, 1, 4)
    sh["w1"] = np.ascontiguousarray(w1.reshape(L, NF, P, NE * P)).astype(nbf16)
    sh["b1"] = np.ascontiguousarray(
        g("b1").astype(np.float32).reshape(L, NF, P).transpose(0, 2, 1))

    W2T = g("W2").transpose(0, 2, 1)  # [L,F,E]
    sh["w2"] = np.ascontiguousarray(W2T.reshape(L, NF, P, E)).astype(nbf16)
    sh["b2"] = g("b2").astype(nbf16)

    for ks, kd in [("ln1_s", "ln1s"), ("ln1_b", "ln1b"),
                   ("ln2_s", "ln2s"), ("ln2_b", "ln2b")]:
        sh[kd] = g(ks).astype(np.float32)
    sh["lnfs"] = g("lnf_s").astype(np.float32)
    sh["lnfb"] = g("lnf_b").astype(np.float32)

    WgT = np.asarray(g("Wg")).T  # [E,V]
    sh["wg"] = np.ascontiguousarray(WgT.reshape(NE, P, V)).astype(nbf16)
    sh["bg"] = g("bg").astype(nbf16)
    return sh


def prep_bias(inputs):
    """[B,S,S,H] gathered bias -> [B, 2, P, H, S] bf16 with causal/pad masks."""
    dist = np.asarray(inputs["distance_squares"]).astype(np.int64)
    isop = np.asarray(inputs["isopen_squares"]).astype(np.int64)
    de = np.asarray(inputs["dist_emb"]).astype(np.float32)[dist]    # [B,S,S,H]
    ie = np.asarray(inputs["isopen_emb"]).astype(np.float32)[isop]  # [B,S,S,H]
    bias = de + ie
    causal = np.tril(np.ones((S, S), bool))
    bias = np.where(causal[None, :, :, None], bias, NEG)
    pad_id = int(np.asarray(inputs["pad_id"]))
    kpm = np.asarray(inputs["sequences"]) == pad_id                 # [B,S]
    bias = np.where(kpm[:, None, :, None], NEG, bias)
    bias = bias.transpose(0, 1, 3, 2)                               # [B,q,H,k]
    return np.ascontiguousarray(bias.reshape(B, 2, P, H, S)).astype(nbf16)


def make_in_maps(inputs):
    sh = prep_shared(inputs)
    bias = prep_bias(inputs)
    seq = np.asarray(inputs["sequences"]).astype(np.int32)  # [B,S]
    in_maps = []
    for c in range(NCORES):
        m = dict(sh)
        m["seq"] = np.ascontiguousarray(seq[c * BL:(c + 1) * BL].reshape(T))
        m["bias"] = np.ascontiguousarray(bias[c * BL:(c + 1) * BL])
        in_maps.append(m)
    return in_maps


LAST_RES = None


def kernel(**inputs):
    global LAST_RES
    nc = build()
    in_maps = make_in_maps(inputs)
    res = run_bass_kernel_spmd(nc, in_maps, core_ids=list(range(NCORES)))
    LAST_RES = res
    out = np.concatenate(
        [np.asarray(r["out"]).reshape(BL, S, V) for r in res.results], axis=0)
    return out.astype(np.float32)

